# revision 10
# baseline (speedup 1.0000x reference)
"""Trainium2 Bass kernel for nn_CausalSelfAttention_43009802502282.

Causal self-attention with external memory (mem_k == mem_v), B=4, T=2048,
C=1024, 16 heads x 64, MEM=256.

Sharding (8 cores): core c -> batch b = c//2, head-group g = c%2 (8 heads).
Each core computes qkv for its heads (column-sliced W_attn), attention for
its 8 heads, and a partial projection (row-sliced W_proj). Host sums the
two partials per batch and adds b_proj.

v2: all-bf16 matmul pipeline (host converts inputs), everything resident in
SBUF (single prologue DMA pass), causal mask applied post-exp as a DVE 0/1
multiply, qk bias folded into the PSUM->SBUF copy (tensor_scalar), PSUM
partitioned 4(sc)+2(av)+2(filler) banks so qkv/proj matmuls fill PE gaps
while ScalarE drains the exp stream. Scores: k-major [keys=128, 2 heads x
512 q] tiles; AV contracts K=128 with a ones-column on v (M=65) so the
softmax denominator falls out of the same matmul.
"""
import sys

sys.path.insert(0, "/opt/trn_rl_repo")

import numpy as np
import ml_dtypes
import concourse.bass as bass
import concourse.mybir as mybir
from concourse import bacc
from concourse.tile import TileContext
from concourse.bass_utils import run_bass_kernel_spmd

F32 = mybir.dt.float32
BF16 = mybir.dt.bfloat16
EXP = mybir.ActivationFunctionType.Exp

B, T, C = 4, 2048, 1024
NH, D, MEM = 16, 64, 256
HL = 8                        # heads per core
P = 128
S = MEM + T                   # 2304
NST = S // P                  # 18 s-tiles (0,1 = memory; 2..17 causal)
NKT = C // P                  # 8 contraction tiles
SCALE = 0.125                 # 1/sqrt(64)
BF = ml_dtypes.bfloat16

_CACHE: dict = {}


def _build(num_devices=8, reps=1):
    nc = bacc.Bacc("TRN2", target_bir_lowering=False, debug=False, num_devices=num_devices)

    xT_d = nc.dram_tensor("xT", [C, T], BF16, kind="ExternalInput")
    memT_d = nc.dram_tensor("memT", [HL * D, MEM], BF16, kind="ExternalInput")
    memn_d = nc.dram_tensor("memn", [MEM, HL * D], BF16, kind="ExternalInput")
    wqk_d = nc.dram_tensor("wqk", [C, 1024], BF16, kind="ExternalInput")
    wv_d = nc.dram_tensor("wv", [C, 512], BF16, kind="ExternalInput")
    wp_d = nc.dram_tensor("wp", [512, C], BF16, kind="ExternalInput")
    bqk_d = nc.dram_tensor("bqk", [P, 8], F32, kind="ExternalInput")
    bvb_d = nc.dram_tensor("bvb", [P, HL, D], BF16, kind="ExternalInput")
    masks_d = nc.dram_tensor("masks", [4, P, 512], BF16, kind="ExternalInput")
    out_d = nc.dram_tensor("out", [T, C], F32, kind="ExternalOutput")

    import contextlib

    with TileContext(nc) as tc:
        with (tc.For_i(0, reps, 1) if reps > 1 else contextlib.nullcontext()):
            _body(nc, tc, dict(locals()))
    nc.compile()
    return nc


def _body(nc, tc, dr):
    xT_d, memT_d, memn_d = dr["xT_d"], dr["memT_d"], dr["memn_d"]
    wqk_d, wv_d, wp_d = dr["wqk_d"], dr["wv_d"], dr["wp_d"]
    bqk_d, bvb_d, masks_d = dr["bqk_d"], dr["bvb_d"], dr["masks_d"]
    out_d = dr["out_d"]
    AluOp = mybir.AluOpType

    with tc.tile_pool(name="pers", bufs=1) as pers, \
         tc.tile_pool(name="expp", bufs=6) as expp, \
         tc.tile_pool(name="stgp", bufs=4) as stgp, \
         tc.tile_pool(name="stkp", bufs=14) as stkp, \
         tc.tile_pool(name="dnp", bufs=2) as dnp, \
         tc.tile_pool(name="outp", bufs=3) as outp, \
         tc.tile_pool(name="scps", bufs=2, space="PSUM") as scps, \
         tc.tile_pool(name="avps", bufs=2, space="PSUM") as avps, \
         tc.tile_pool(name="flps", bufs=2, space="PSUM") as flps:

        xT_s = pers.tile([P, NKT, T], BF16, tag="xT")
        wqk_s = pers.tile([P, NKT, 1024], BF16, tag="wqk")
        wv_s = pers.tile([P, NKT, 512], BF16, tag="wv")
        wp_s = pers.tile([P, 4, C], BF16, tag="wp")
        qkT = pers.tile([P, 8, T], BF16, tag="qkT")       # rows: q (mt 0-3), k (mt 4-7)
        v_s = pers.tile([P, NST, HL, D + 1], BF16, tag="v")
        memT_s = pers.tile([P, 4, MEM], BF16, tag="memT")
        masks_s = pers.tile([P, 4, 512], BF16, tag="masks")
        bqk_s = pers.tile([P, 8], F32, tag="bqk")
        bvb_s = pers.tile([P, HL, D], BF16, tag="bvb")

        # prologue DMAs: wqk + x(ch0) first so qkv matmuls can start early
        for c in range(NKT):
            eng = nc.sync if c % 2 == 0 else nc.gpsimd
            eng2 = nc.gpsimd if c % 2 == 0 else nc.sync
            eng.dma_start(wqk_s[:, c], wqk_d[128 * c:128 * c + 128, :])
            eng2.dma_start(xT_s[:, c, 0:512], xT_d[128 * c:128 * c + 128, 0:512])
        nc.sync.dma_start(memT_s[:], memT_d[:, :].rearrange("(rt p) m -> p rt m", p=P))
        nc.sync.dma_start(bqk_s[:], bqk_d[:, :])
        for c in range(NKT):
            eng = nc.sync if c % 2 == 0 else nc.gpsimd
            eng2 = nc.gpsimd if c % 2 == 0 else nc.sync
            eng.dma_start(wv_s[:, c], wv_d[128 * c:128 * c + 128, :])
            eng2.dma_start(xT_s[:, c, 512:T], xT_d[128 * c:128 * c + 128, 512:T])
        nc.gpsimd.dma_start(wp_s[:], wp_d[:, :].rearrange("(rt p) c -> p rt c", p=P))
        for st in range(2):
            nc.sync.dma_start(v_s[:, st, :, 0:D],
                              memn_d[128 * st:128 * st + 128, :]
                              .rearrange("p (j d) -> p j d", d=D))
        nc.vector.memset(v_s[:, :, :, D], 1.0)
        nc.sync.dma_start(masks_s[:], masks_d[:, :, :].rearrange("m p t -> p m t"))
        nc.gpsimd.dma_start(bvb_s[:], bvb_d[:, :, :])

        # filler units are split into 4-matmul granules so a pending filler
        # block never delays the next strip's QK by more than ~0.5us
        def qk_granules(ch, mt):
            cell = {}

            def g0():
                cell["ps"] = flps.tile([P, 512], F32, tag="fl", name=f"qk_{ch}_{mt}")
                for c in range(4):
                    nc.tensor.matmul(cell["ps"][:], wqk_s[:, c, 128 * mt:128 * mt + 128],
                                     xT_s[:, c, 512 * ch:512 * ch + 512],
                                     start=(c == 0), stop=False)

            def g1():
                ps = cell["ps"]
                for c in range(4, NKT):
                    nc.tensor.matmul(ps[:], wqk_s[:, c, 128 * mt:128 * mt + 128],
                                     xT_s[:, c, 512 * ch:512 * ch + 512],
                                     start=False, stop=(c == NKT - 1))
                with nc.allow_low_precision(reason="bf16 qkv"):
                    nc.vector.tensor_scalar(qkT[:, mt, 512 * ch:512 * ch + 512], ps[:],
                                            bqk_s[:, mt:mt + 1], None, AluOp.add)

            return [g0, g1]

        def v_granules(ch, tl):
            st = 2 + 4 * ch + tl
            cell = {}

            def g0():
                cell["ps"] = flps.tile([P, HL, D], F32, tag="fl", name=f"v_{st}")
                for c in range(4):
                    nc.tensor.matmul(cell["ps"][:],
                                     xT_s[:, c, 512 * ch + 128 * tl:512 * ch + 128 * tl + 128],
                                     wv_s[:, c], start=(c == 0), stop=False)

            def g1():
                ps = cell["ps"]
                for c in range(4, NKT):
                    nc.tensor.matmul(ps[:],
                                     xT_s[:, c, 512 * ch + 128 * tl:512 * ch + 128 * tl + 128],
                                     wv_s[:, c], start=False, stop=(c == NKT - 1))
                with nc.allow_low_precision(reason="bf16 qkv"):
                    nc.vector.tensor_tensor(v_s[:, st, :, 0:D], ps[:], bvb_s[:], AluOp.add)

            return [g0, g1]

        def qkv_units(ch, mts=range(8), tls=range(4)):
            out = []
            for mt in mts:
                out += qk_granules(ch, mt)
            for tl in tls:
                out += v_granules(ch, tl)
            return out

        def emit_attn(ch, filler):
            n_st = 6 + 4 * ch
            strips_total = 4 * n_st
            kstrip, emitted = 0, 0
            stks = []
            for pr in range(4):
                j0, j1 = 2 * pr, 2 * pr + 1
                rt = pr
                qT0 = qkT[0:64, rt, 512 * ch:512 * ch + 512]
                qT1 = qkT[64:128, rt, 512 * ch:512 * ch + 512]
                av0 = avps.tile([D + 1, 512], F32, tag="av", name=f"av0_{ch}_{pr}")
                av1 = avps.tile([D + 1, 512], F32, tag="av", name=f"av1_{ch}_{pr}")
                for st in range(n_st):
                    diag = st >= 2 + 4 * ch
                    sp = st - 2 - 4 * ch if diag else 0
                    vs = 128 * sp
                    if st < 2:
                        kf0 = memT_s[0:64, rt, 128 * st:128 * st + 128]
                        kf1 = memT_s[64:128, rt, 128 * st:128 * st + 128]
                    else:
                        kf0 = qkT[0:64, 4 + rt, 128 * (st - 2):128 * (st - 2) + 128]
                        kf1 = qkT[64:128, 4 + rt, 128 * (st - 2):128 * (st - 2) + 128]
                    sc = scps.tile([P, 2, 512], F32, tag="sc", name=f"sc_{ch}_{pr}_{st}")
                    nc.tensor.matmul(sc[:, 0, vs:512], kf0, qT0[:, vs:512],
                                     start=True, stop=True)
                    nc.tensor.matmul(sc[:, 1, vs:512], kf1, qT1[:, vs:512],
                                     start=True, stop=True)
                    ex = expp.tile([P, 2, 512], BF16, tag="ex", name=f"ex_{ch}_{pr}_{st}")
                    with nc.allow_low_precision(reason="bf16 attention probs"):
                        nc.scalar.activation(ex[:, :, vs:512], sc[:, :, vs:512], EXP,
                                             scale=SCALE)
                        if diag:
                            nc.vector.tensor_tensor(ex[:, 0, vs:512], ex[:, 0, vs:512],
                                                    masks_s[:, sp, vs:512], AluOp.mult)
                            nc.vector.tensor_tensor(ex[:, 1, vs:512], ex[:, 1, vs:512],
                                                    masks_s[:, sp, vs:512], AluOp.mult)
                    nc.tensor.matmul(av0[:, vs:512], v_s[:, st, j0], ex[:, 0, vs:512],
                                     start=(st == 0), stop=(st == n_st - 1))
                    nc.tensor.matmul(av1[:, vs:512], v_s[:, st, j1], ex[:, 1, vs:512],
                                     start=(st == 0), stop=(st == n_st - 1))
                    kstrip += 1
                    want = len(filler) * kstrip // strips_total
                    while emitted < want:
                        filler[emitted]()
                        emitted += 1

                stg0 = stgp.tile([D + 1, 512], F32, tag="stg", name=f"stg0_{ch}_{pr}")
                stg1 = stgp.tile([D + 1, 512], F32, tag="stg", name=f"stg1_{ch}_{pr}")
                nc.vector.tensor_copy(stg0[:], av0[:])
                nc.vector.tensor_copy(stg1[:], av1[:])
                # denominator rows live at partition 64; partition_broadcast
                # reads absolute partition 0, so DMA-shift them first.
                denr = dnp.tile([1, 2, 512], F32, tag="denr", name=f"dn_{ch}_{pr}")
                nc.sync.dma_start(denr[:, 0], stg0[D:D + 1, :])
                nc.sync.dma_start(denr[:, 1], stg1[D:D + 1, :])
                rec = dnp.tile([1, 2, 512], F32, tag="rec", name=f"rc_{ch}_{pr}")
                with nc.allow_low_precision(reason="softmax denom reciprocal"):
                    nc.vector.reciprocal(rec[:], denr[:])
                bcs = dnp.tile([D, 2, 512], F32, tag="bcs", name=f"bc_{ch}_{pr}")
                nc.gpsimd.partition_broadcast(bcs[:], rec[:])
                stk = stkp.tile([P, 512], BF16, tag="stk", name=f"stk_{ch}_{pr}")
                stk1t = stgp.tile([D, 512], BF16, tag="stk1t", name=f"s1t_{ch}_{pr}")
                with nc.allow_low_precision(reason="bf16 y"):
                    nc.vector.tensor_tensor(stk[0:D, :], stg0[0:D, :], bcs[:, 0],
                                            AluOp.mult)
                    nc.vector.tensor_tensor(stk1t[:], stg1[0:D, :], bcs[:, 1],
                                            AluOp.mult)
                # partition-shift the odd head into rows 64..127
                nc.sync.dma_start(stk[D:2 * D, :], stk1t[:])
                stks.append(stk)
            while emitted < len(filler):
                filler[emitted]()
                emitted += 1
            return stks

        def proj_units(ch, stks):
            def granules(mt, n2):
                cell = {}

                def g(pr):
                    def emit():
                        if pr == 0:
                            cell["pp"] = flps.tile([P, 512], F32, tag="fl",
                                                   name=f"pj_{ch}_{mt}_{n2}")
                        pp = cell["pp"]
                        nc.tensor.matmul(pp[:], stks[pr][:, 128 * mt:128 * mt + 128],
                                         wp_s[:, pr, 512 * n2:512 * n2 + 512],
                                         start=(pr == 0), stop=(pr == 3))
                        if pr == 3:
                            ot = outp.tile([P, 512], F32, tag="ot",
                                           name=f"ot_{ch}_{mt}_{n2}")
                            nc.vector.tensor_copy(ot[:], pp[:])
                            oeng = nc.sync if (mt + n2) % 2 == 0 else nc.gpsimd
                            oeng.dma_start(
                                out_d[512 * ch + 128 * mt:512 * ch + 128 * mt + 128,
                                      512 * n2:512 * n2 + 512], ot[:])
                    return emit

                return [g(pr) for pr in range(4)]

            out = []
            for mt in range(4):
                for n2 in range(2):
                    out += granules(mt, n2)
            return out

        # Filler placement: qkv(ch+1) must drain during attn(ch) (attn(ch+1)
        # reads it), but attn(3) only needs qkv(3)'s q/k-units per pr block
        # and v-units per diag strip, so its later k/v units plus deferred
        # proj(1)/proj(2) cover attn(3)'s ACT-over-PE deficit.
        for u in qkv_units(0):
            u()
        stks0 = emit_attn(0, qkv_units(1))
        stks1 = emit_attn(1, qkv_units(2))
        stks2 = emit_attn(2, qkv_units(3, mts=range(5), tls=range(4))
                          + proj_units(0, stks0))
        stks3 = emit_attn(3, qkv_units(3, mts=range(5, 8), tls=[])
                          + proj_units(1, stks1) + proj_units(2, stks2))
        for u in proj_units(3, stks3):
            u()


def _host_inputs(x, ext_mem, W_attn, b_attn, W_proj, b_proj):
    """Per-core input maps (host-side sharding/layout prep, no FLOPs)."""
    masks = np.zeros((4, P, 512), dtype=BF)
    pp_ = np.arange(P)[:, None]
    tt_ = np.arange(512)[None, :]
    for sp in range(4):
        masks[sp] = np.where(pp_ + 128 * sp <= tt_, 1.0, 0.0).astype(BF)

    in_maps = []
    for c in range(8):
        b, g = c // 2, c % 2
        qs = slice(512 * g, 512 * g + 512)
        ks = slice(1024 + 512 * g, 1024 + 512 * g + 512)
        vs = slice(2048 + 512 * g, 2048 + 512 * g + 512)
        mem = np.ascontiguousarray(ext_mem[b][:, 512 * g:512 * g + 512])
        bqk_full = np.concatenate([b_attn[qs], b_attn[ks]]).astype(np.float32)
        in_maps.append({
            "xT": np.ascontiguousarray(x[b].T).astype(BF),
            "memT": np.ascontiguousarray(mem.T).astype(BF),
            "memn": mem.astype(BF),
            "wqk": np.ascontiguousarray(
                np.concatenate([W_attn[:, qs], W_attn[:, ks]], axis=1)).astype(BF),
            "wv": np.ascontiguousarray(W_attn[:, vs]).astype(BF),
            "wp": np.ascontiguousarray(W_proj[512 * g:512 * g + 512, :]).astype(BF),
            "bqk": np.ascontiguousarray(bqk_full.reshape(8, 128).T),
            "bvb": np.broadcast_to(
                b_attn[vs].reshape(HL, D)[None], (P, HL, D)).astype(BF),
            "masks": masks,
        })
    return in_maps


def kernel(x, ext_mem, W_attn, b_attn, W_proj, b_proj):
    x = np.asarray(x, dtype=np.float32)
    ext_mem = np.asarray(ext_mem, dtype=np.float32)
    W_attn = np.asarray(W_attn, dtype=np.float32)
    b_attn = np.asarray(b_attn, dtype=np.float32)
    W_proj = np.asarray(W_proj, dtype=np.float32)
    b_proj = np.asarray(b_proj, dtype=np.float32)

    if "nc" not in _CACHE:
        _CACHE["nc"] = _build()
    nc = _CACHE["nc"]

    in_maps = _host_inputs(x, ext_mem, W_attn, b_attn, W_proj, b_proj)
    res = run_bass_kernel_spmd(nc, in_maps, list(range(8)))

    out = np.empty((B, T, C), dtype=np.float32)
    for b in range(B):
        out[b] = res.results[2 * b]["out"] + res.results[2 * b + 1]["out"] + b_proj
    return out
